# revision 15
# baseline (speedup 1.0000x reference)
"""Trainium2 Bass kernel for nn_CompositionBlock (gnn_message_passing).

Data-parallel over batch B=8 across 8 NeuronCores (one sample per core).

U-first bilinear formulation: for each token j the bilinears are
  tde[p,j] = sum_{t,d} W1[p,t,d] * (tok[j,t]*dep[j,d])
  comp[o,j] = sum_{t,p} W2[o,t,p] * (tok[j,t]*h[p,j])
The elementwise outer-product operands U/V are built by DVE from
host-replicated f16 tiles (all-SBUF 16-bit => DVE fast mode), and each
bilinear is ONE PSUM accumulation chain of K-chunk matmuls (no selection
-matrix reduction matmuls).  h replication across partitions is done on
the PE with a selection matrix.  Head-scatter stays a one-hot matmul.
"""

import json

import numpy as np

B, S, T, D, P = 8, 256, 128, 64, 128
NCORES = 8
JT = S // 128  # token tiles (j) per core


# ----------------------------------------------------------------------------
# Compat: the walrus build in this container accepts at most one sync-wait on
# CTRL-class instructions, but TileContext's tail drain packs several. Split
# any multi-wait instruction into a chain of single-wait clones.
# ----------------------------------------------------------------------------
def _split_multiwait_bir(bir_json_bytes: bytes) -> bytes:
    bir = json.loads(bir_json_bytes)
    for func in bir.get("functions", []):
        for bb in func.get("blocks", []):
            new_instructions = []
            for ins in bb.get("instructions", []):
                si = ins.get("sync_info") or {}
                waits = si.get("on_wait") or []
                if len(waits) > 1:
                    for i, w in enumerate(waits[:-1]):
                        new_instructions.append({
                            "debug": ins.get("debug", 0),
                            "engine": ins["engine"],
                            "ins": [],
                            "name": f"{ins['name']}_w{i}",
                            "opcode": "NoOp",
                            "outs": [],
                            "sync_info": {"on_wait": [w], "on_update": []},
                        })
                    ins["sync_info"] = {
                        "on_wait": [waits[-1]],
                        "on_update": si.get("on_update") or [],
                    }
                new_instructions.append(ins)
            bb["instructions"] = new_instructions
    return json.dumps(bir).encode()


def _install_compat():
    import concourse.bass_utils as bu

    if getattr(bu.compile_bir_kernel, "_multiwait_patched", False):
        return
    orig = bu.compile_bir_kernel

    def patched(bir_json, tmpdir, neff_name="file.neff"):
        return orig(_split_multiwait_bir(bir_json), tmpdir, neff_name)

    patched._multiwait_patched = True
    bu.compile_bir_kernel = patched
    try:
        import concourse.bass2jax as b2j

        if getattr(b2j, "compile_bir_kernel", None) is not None:
            b2j.compile_bir_kernel = patched
    except ImportError:
        pass


_NC_CACHE = {}


def build_nc():
    if "nc" in _NC_CACHE:
        return _NC_CACHE["nc"]
    import concourse.bass as bass
    import concourse.tile as tile
    from concourse import mybir
    from concourse.masks import make_identity

    f32 = mybir.dt.float32
    f16 = mybir.dt.float16
    Alu = mybir.AluOpType
    Act = mybir.ActivationFunctionType

    nc = bass.Bass(trn_type="TRN2")

    # DRAM inputs (all host-prepped into final SBUF layouts)
    tok16_d = nc.dram_tensor("tok16", [4, 128, 1024], f16, kind="ExternalInput")
    dep8_d = nc.dram_tensor("dep8", [128, 1024], f16, kind="ExternalInput")
    w1_d = nc.dram_tensor("w1", [4, 128, 2048], f16, kind="ExternalInput")
    w2_d = nc.dram_tensor("w2", [8, 128, 2048], f16, kind="ExternalInput")
    rep_d = nc.dram_tensor("rep", [128, 1024], f16, kind="ExternalInput")
    iota_d = nc.dram_tensor("iota", [1, S], f16, kind="ExternalInput")
    cpk_d = nc.dram_tensor("cpk", [128, 8], f32, kind="ExternalInput")
    c0_d = nc.dram_tensor("c0", [1, T], f16, kind="ExternalInput")
    one_d = nc.dram_tensor("one", [1, 128], f16, kind="ExternalInput")
    out_d = nc.dram_tensor("out", [S, T], f32, kind="ExternalOutput")

    def bcast_row(dram_ap):
        return bass.AP(
            tensor=dram_ap.tensor,
            offset=dram_ap.offset,
            ap=[[0, 128]] + list(dram_ap.ap[1:]),
        )

    with tile.TileContext(nc) as tc:
        with (
            tc.tile_pool(name="consts", bufs=1) as consts,
            tc.tile_pool(name="weights", bufs=1) as weights,
            tc.tile_pool(name="acts", bufs=1) as acts,
            tc.tile_pool(name="up", bufs=3) as up,
            tc.tile_pool(name="h8p", bufs=3) as h8p,
            tc.tile_pool(name="vp", bufs=3) as vp,
            tc.tile_pool(name="pstde", bufs=1, space="PSUM") as pstde,
            tc.tile_pool(name="psrep", bufs=2, space="PSUM") as psrep,
            tc.tile_pool(name="pscomp", bufs=1, space="PSUM") as pscomp,
            tc.tile_pool(name="psmm", bufs=2, space="PSUM") as psmm,
            tc.tile_pool(name="psfin", bufs=1, space="PSUM") as psfin,
        ):
            ident16 = consts.tile([128, 128], f16)
            make_identity(nc, ident16)

            # ---- tiles ----
            tok16_sb = weights.tile([128, 4096], f16)
            dep8_sb = weights.tile([128, 1024], f16)
            w1_sb = [
                weights.tile([128, 2048], f16, name=f"w1_{q}", tag=f"w1_{q}")
                for q in range(4)
            ]
            rep_sb = weights.tile([128, 1024], f16)
            w2_sb = [
                weights.tile([128, 2048], f16, name=f"w2_{pb}", tag=f"w2_{pb}")
                for pb in range(8)
            ]
            cpk = consts.tile([128, 8], f32)
            iota_b = consts.tile([128, S], f16)
            c0row = consts.tile([1, T], f16)
            onerow = consts.tile([1, 128], f16)
            bdep_c = cpk[:, 0:1]
            bcomp_c = cpk[:, 1:2]
            base_c = cpk[:, 2:3]
            headsf_t = [cpk[:, 3:4], cpk[:, 4:5]]
            wr_t = [cpk[:, 5:6], cpk[:, 6:7]]

            # ---- DMAs: strict consumption order, two HW DGE queues ----
            # sync queue
            nc.sync.dma_start(out=tok16_sb[:, 0:1024], in_=tok16_d[0, :, :])
            nc.sync.dma_start(out=tok16_sb[:, 1024:2048], in_=tok16_d[1, :, :])
            nc.sync.dma_start(out=w1_sb[0], in_=w1_d[0, :, :])
            nc.sync.dma_start(out=w1_sb[1], in_=w1_d[1, :, :])
            for pb in (0, 2, 4, 6, 5, 7):
                nc.sync.dma_start(out=w2_sb[pb], in_=w2_d[pb, :, :])
            nc.sync.dma_start(out=c0row, in_=c0_d[:, :])
            nc.sync.dma_start(out=onerow, in_=one_d[:, :])
            # scalar queue
            nc.scalar.dma_start(out=dep8_sb, in_=dep8_d[:, :])
            nc.scalar.dma_start(out=tok16_sb[:, 2048:3072], in_=tok16_d[2, :, :])
            nc.scalar.dma_start(out=tok16_sb[:, 3072:4096], in_=tok16_d[3, :, :])
            nc.scalar.dma_start(out=w1_sb[2], in_=w1_d[2, :, :])
            nc.scalar.dma_start(out=w1_sb[3], in_=w1_d[3, :, :])
            nc.scalar.dma_start(out=cpk, in_=cpk_d[:, :])
            nc.scalar.dma_start(out=rep_sb, in_=rep_d[:, :])
            for pb in (1, 3):
                nc.scalar.dma_start(out=w2_sb[pb], in_=w2_d[pb, :, :])
            nc.scalar.dma_start(out=iota_b, in_=bcast_row(iota_d[:, :]))

            # preload tanh act table off the critical path
            warm = consts.tile([128, 1], f32)
            nc.scalar.activation(warm, cpk[:, 7:8], Act.Tanh)

            # ---- stage 1: tde[p, j], 64 K-chunks, one fused U op per q ----
            tde_ps = pstde.tile([128, S], f32)
            ci1 = 0
            for q in range(4):
                u = up.tile([128, 4096], f16, name="U", tag="U")
                for db in range(4):
                    # U_q_db[p, (ti, j)] = tok16[p, (q, ti, j)] * dep8[p, (db, j)]
                    out_v = u[:, db * 1024:(db + 1) * 1024] \
                        .rearrange("p (ti j) -> p ti j", j=256)
                    in0_v = tok16_sb[:, q * 1024:(q + 1) * 1024] \
                        .rearrange("p (ti j) -> p ti j", j=256)
                    in1_v = dep8_sb[:, db * 256:(db + 1) * 256] \
                        .unsqueeze(1).broadcast_to([128, 4, 256])
                    nc.vector.scalar_tensor_tensor(
                        out=out_v, in0=in0_v, scalar=1.0, in1=in1_v,
                        op0=Alu.mult, op1=Alu.mult)
                    for ti in range(4):
                        nc.tensor.matmul(
                            tde_ps,
                            w1_sb[q][:, (db * 4 + ti) * 128:(db * 4 + ti + 1) * 128],
                            u[:, (db * 1024) + ti * 256:(db * 1024) + (ti + 1) * 256],
                            start=(ci1 == 0),
                            stop=(ci1 == 63),
                        )
                        ci1 += 1

            # h = tanh(tde + b_dep), f16 [p, j]
            hT = acts.tile([128, S], f16)
            nc.scalar.activation(hT, tde_ps, Act.Tanh, bias=bdep_c)

            # ---- stage 2: comp[o, j], 128 K-chunks, one fused V op per pb ----
            comp_ps = pscomp.tile([128, S], f32)
            rep_ps = {}

            def emit_rep(pb):
                ps = psrep.tile([128, S], f32, name="rep_ps", tag="rep_ps")
                nc.tensor.matmul(
                    ps, rep_sb[:, pb * 128:(pb + 1) * 128], hT,
                    start=True, stop=True,
                )
                rep_ps[pb] = ps

            emit_rep(0)
            for pb in range(8):
                if pb + 1 < 8:
                    emit_rep(pb + 1)
                h8 = h8p.tile([128, S], f16, name="h8", tag="h8")
                nc.scalar.copy(h8, rep_ps.pop(pb))
                v = vp.tile([128, 4096], f16, name="V", tag="V")
                out_v = v[:, :].rearrange("p (tb j) -> p tb j", j=256)
                in0_v = tok16_sb[:, :].rearrange("p (tb j) -> p tb j", j=256)
                in1_v = h8[:, :].unsqueeze(1).broadcast_to([128, 16, 256])
                nc.vector.scalar_tensor_tensor(
                    out=out_v, in0=in0_v, scalar=1.0, in1=in1_v,
                    op0=Alu.mult, op1=Alu.mult)
                for tb in range(16):
                    ci2 = pb * 16 + tb
                    nc.tensor.matmul(
                        comp_ps,
                        w2_sb[pb][:, tb * 128:(tb + 1) * 128],
                        v[:, tb * 256:(tb + 1) * 256],
                        start=(ci2 == 0),
                        stop=(ci2 == 127),
                    )

            # soh[jt][j, i] = (head[j] == i) * wr[j], f16
            soh = []
            for jt in range(JT):
                s = acts.tile([128, S], f16, name=f"soh{jt}", tag=f"soh{jt}")
                nc.vector.tensor_scalar(
                    out=s, in0=iota_b, scalar1=headsf_t[jt], scalar2=wr_t[jt],
                    op0=Alu.is_equal, op1=Alu.mult,
                )
                soh.append(s)

            # spec = tanh(comp + b_comp); delta = spec - base  (f16, [o, j])
            specT = acts.tile([128, S], f32)
            nc.scalar.activation(specT, comp_ps, Act.Tanh, bias=bcomp_c)
            deltaT = acts.tile([128, S], f16)
            nc.vector.tensor_scalar(
                out=deltaT, in0=specT, scalar1=base_c, scalar2=None,
                op0=Alu.subtract,
            )

            # transpose deltaT -> delta[j, o] per token tile
            delta_sb = []
            for jt in range(JT):
                dps = psmm.tile([128, 128], f16, name="dps", tag="dps")
                nc.tensor.transpose(
                    dps, deltaT[:, jt * 128:(jt + 1) * 128], ident16
                )
                dsb = acts.tile([128, 128], f16, name=f"delta{jt}", tag=f"delta{jt}")
                nc.scalar.copy(dsb, dps)
                delta_sb.append(dsb)

            # fin[i, o] = sum_jt soh[jt][:, i-chunk].T @ delta[jt]  (+ c0 via
            # a K=1 rank-1 matmul: ones[1,i] x c0row[1,o])
            fin_ps = psfin.tile([128, S], f32)
            for ic in range(2):
                for jt in range(JT):
                    nc.tensor.matmul(
                        fin_ps[:, ic * 128:(ic + 1) * 128],
                        soh[jt][:, ic * 128:(ic + 1) * 128],
                        delta_sb[jt],
                        start=(jt == 0),
                        stop=False,
                    )
                nc.tensor.matmul(
                    fin_ps[:, ic * 128:(ic + 1) * 128],
                    onerow, c0row,
                    start=False, stop=True,
                )
            for ic in range(2):
                outsb = acts.tile([128, T], f32, name="outsb", tag=f"outsb{ic}")
                nc.scalar.copy(outsb, fin_ps[:, ic * 128:(ic + 1) * 128])
                nc.sync.dma_start(
                    out=out_d[ic * 128:(ic + 1) * 128, :], in_=outsb
                )

    _NC_CACHE["nc"] = nc
    return nc


def prep_core_inputs(token_embeddings, dep_embeddings, dep_heads,
                     W_dep, b_dep, W_comp, b_comp, W_red, b_red):
    f32 = np.float32
    f16 = np.float16
    tok = np.asarray(token_embeddings, dtype=f32)
    dep = np.asarray(dep_embeddings, dtype=f32)
    heads = np.asarray(dep_heads)
    W_dep = np.asarray(W_dep, dtype=f32)
    b_dep = np.asarray(b_dep, dtype=f32)
    W_comp = np.asarray(W_comp, dtype=f32)
    b_comp = np.asarray(b_comp, dtype=f32)
    wr = np.asarray(W_red, dtype=f32)[0]
    b_red = np.asarray(b_red, dtype=f32)

    # W1sb[(d',t'), (q, db, ti, p)] = W_dep[p, 8*(4q+ti)+t', 16*db+d']
    A = W_dep.reshape(P, 4, 4, 8, 4, 16)          # [p, q, ti, t', db, d']
    w1 = np.ascontiguousarray(
        A.transpose(5, 3, 1, 4, 2, 0).reshape(128, 4, 2048).transpose(1, 0, 2)
    ).astype(f16)                                 # [q, 128, 2048]

    # W2sb[(p',t'), (pb, tb, o)] = W_comp[o, 8*tb+t', 16*pb+p']
    Bm = W_comp.reshape(T, 16, 8, 8, 16)          # [o, tb, t', pb, p']
    w2 = np.ascontiguousarray(
        Bm.transpose(4, 2, 3, 1, 0).reshape(128, 8, 2048).transpose(1, 0, 2)
    ).astype(f16)                                 # [pb, 128, 2048]

    # rep[k, (pb, r)] = 1 if k == 16*pb + r//8
    rep = np.zeros((128, 8, 128), dtype=f16)
    r_ = np.arange(128)
    for pb in range(8):
        rep[16 * pb + r_ // 8, pb, r_] = 1.0
    rep = rep.reshape(128, 1024)

    base = np.tanh(b_comp)
    c0 = (base * wr.sum() + b_red[0]).astype(f32)
    iota = np.arange(S, dtype=f16).reshape(1, S)
    headsf = heads.astype(f32).reshape(B, JT, 128)
    wr_t = np.ascontiguousarray(wr.reshape(JT, 128, 1))

    cpk = np.zeros((128, 8), dtype=f32)
    cpk[:, 0] = b_dep
    cpk[:, 1] = b_comp
    cpk[:, 2] = base
    cpk[:, 5] = wr[:128]
    cpk[:, 6] = wr[128:]
    shared = {
        "w1": w1, "w2": w2, "rep": rep,
        "iota": iota,
        "c0": c0.reshape(1, T).astype(f16),
        "one": np.ones((1, 128), dtype=f16),
    }
    in_maps = []
    for c in range(NCORES):
        # tok16[(rep16, t'), (tb, j)] = tok[c][j, 8*tb + t']
        tokT3 = np.ascontiguousarray(tok[c].T).reshape(16, 8, S)   # [tb, t', j]
        tmp = tokT3.transpose(1, 0, 2)                             # [t', tb, j]
        tok16 = np.ascontiguousarray(
            np.broadcast_to(tmp[None], (16, 8, 16, S))
            .reshape(128, 4, 1024).transpose(1, 0, 2)
        ).astype(f16)                                              # [q, 128, 1024]
        # dep8[(d', rep8), (db, j)] = dep[c][j, 16*db + d']
        depT3 = np.ascontiguousarray(dep[c].T).reshape(4, 16, S)   # [db, d', j]
        dmp = depT3.transpose(1, 0, 2)                             # [d', db, j]
        dep8 = np.ascontiguousarray(
            np.broadcast_to(dmp[:, None], (16, 8, 4, S)).reshape(128, 4 * S)
        ).astype(f16)
        m = dict(shared)
        m["tok16"] = tok16
        m["dep8"] = dep8
        cpkc = cpk.copy()
        cpkc[:, 3] = headsf[c, 0]
        cpkc[:, 4] = headsf[c, 1]
        m["cpk"] = cpkc
        in_maps.append(m)
    return in_maps


def kernel(**inputs) -> np.ndarray:
    _install_compat()
    from concourse.bass_utils import run_bass_kernel_spmd

    nc = build_nc()
    in_maps = prep_core_inputs(**inputs)
    res = run_bass_kernel_spmd(nc, in_maps, core_ids=list(range(NCORES)))
    out = np.stack([res.results[c]["out"] for c in range(NCORES)], axis=0)
    return out.astype(np.float32)


# aliases used by test harness
_build_nc = build_nc
_prep_core_inputs = prep_core_inputs


# revision 16
# speedup vs baseline: 1.2809x; 1.2809x over previous
"""Trainium2 Bass kernel for nn_CompositionBlock (gnn_message_passing).

Data-parallel over batch B=8 across 8 NeuronCores (one sample per core).

U-first bilinear formulation: for each token j the bilinears are
  tde[p,j] = sum_{t,d} W1[p,t,d] * (tok[j,t]*dep[j,d])
  comp[o,j] = sum_{t,p} W2[o,t,p] * (tok[j,t]*h[p,j])
The elementwise outer-product operands U/V are built by DVE from
host-replicated f16 tiles (all-SBUF 16-bit => DVE fast mode), and each
bilinear is ONE PSUM accumulation chain of K-chunk matmuls (no selection
-matrix reduction matmuls).  h replication across partitions is done on
the PE with a selection matrix.  Head-scatter stays a one-hot matmul.
"""

import json

import numpy as np

B, S, T, D, P = 8, 256, 128, 64, 128
NCORES = 8
JT = S // 128  # token tiles (j) per core


# ----------------------------------------------------------------------------
# Compat: the walrus build in this container accepts at most one sync-wait on
# CTRL-class instructions, but TileContext's tail drain packs several. Split
# any multi-wait instruction into a chain of single-wait clones.
# ----------------------------------------------------------------------------
def _split_multiwait_bir(bir_json_bytes: bytes) -> bytes:
    bir = json.loads(bir_json_bytes)
    for func in bir.get("functions", []):
        for bb in func.get("blocks", []):
            new_instructions = []
            for ins in bb.get("instructions", []):
                si = ins.get("sync_info") or {}
                waits = si.get("on_wait") or []
                if len(waits) > 1:
                    for i, w in enumerate(waits[:-1]):
                        new_instructions.append({
                            "debug": ins.get("debug", 0),
                            "engine": ins["engine"],
                            "ins": [],
                            "name": f"{ins['name']}_w{i}",
                            "opcode": "NoOp",
                            "outs": [],
                            "sync_info": {"on_wait": [w], "on_update": []},
                        })
                    ins["sync_info"] = {
                        "on_wait": [waits[-1]],
                        "on_update": si.get("on_update") or [],
                    }
                new_instructions.append(ins)
            bb["instructions"] = new_instructions
    return json.dumps(bir).encode()


def _install_compat():
    import concourse.bass_utils as bu

    if getattr(bu.compile_bir_kernel, "_multiwait_patched", False):
        return
    orig = bu.compile_bir_kernel

    def patched(bir_json, tmpdir, neff_name="file.neff"):
        return orig(_split_multiwait_bir(bir_json), tmpdir, neff_name)

    patched._multiwait_patched = True
    bu.compile_bir_kernel = patched
    try:
        import concourse.bass2jax as b2j

        if getattr(b2j, "compile_bir_kernel", None) is not None:
            b2j.compile_bir_kernel = patched
    except ImportError:
        pass


_NC_CACHE = {}


def build_nc():
    if "nc" in _NC_CACHE:
        return _NC_CACHE["nc"]
    import concourse.bass as bass
    import concourse.tile as tile
    from concourse import mybir
    from concourse.masks import make_identity

    f32 = mybir.dt.float32
    f16 = mybir.dt.float16
    Alu = mybir.AluOpType
    Act = mybir.ActivationFunctionType

    nc = bass.Bass(trn_type="TRN2")

    # DRAM inputs (all host-prepped into final SBUF layouts)
    tok16_d = nc.dram_tensor("tok16", [4, 128, 1024], f16, kind="ExternalInput")
    dep8_d = nc.dram_tensor("dep8", [128, 1024], f16, kind="ExternalInput")
    w1_d = nc.dram_tensor("w1", [4, 128, 2048], f16, kind="ExternalInput")
    w2_d = nc.dram_tensor("w2", [8, 128, 2048], f16, kind="ExternalInput")
    rep_d = nc.dram_tensor("rep", [128, 1024], f16, kind="ExternalInput")
    iota_d = nc.dram_tensor("iota", [1, S], f16, kind="ExternalInput")
    cpk_d = nc.dram_tensor("cpk", [128, 8], f32, kind="ExternalInput")
    c0_d = nc.dram_tensor("c0", [1, T], f16, kind="ExternalInput")
    one_d = nc.dram_tensor("one", [1, 128], f16, kind="ExternalInput")
    out_d = nc.dram_tensor("out", [S, T], f32, kind="ExternalOutput")

    def bcast_row(dram_ap):
        return bass.AP(
            tensor=dram_ap.tensor,
            offset=dram_ap.offset,
            ap=[[0, 128]] + list(dram_ap.ap[1:]),
        )

    with tile.TileContext(nc) as tc:
        with (
            tc.tile_pool(name="consts", bufs=1) as consts,
            tc.tile_pool(name="weights", bufs=1) as weights,
            tc.tile_pool(name="acts", bufs=1) as acts,
            tc.tile_pool(name="up", bufs=3) as up,
            tc.tile_pool(name="h8p", bufs=3) as h8p,
            tc.tile_pool(name="vp", bufs=3) as vp,
            tc.tile_pool(name="pstde", bufs=1, space="PSUM") as pstde,
            tc.tile_pool(name="psrep", bufs=2, space="PSUM") as psrep,
            tc.tile_pool(name="pscomp", bufs=1, space="PSUM") as pscomp,
            tc.tile_pool(name="psmm", bufs=2, space="PSUM") as psmm,
            tc.tile_pool(name="psfin", bufs=1, space="PSUM") as psfin,
        ):
            ident16 = consts.tile([128, 128], f16)
            make_identity(nc, ident16)

            # ---- tiles ----
            tok16_sb = weights.tile([128, 4096], f16)
            dep8_sb = weights.tile([128, 1024], f16)
            w1_sb = [
                weights.tile([128, 2048], f16, name=f"w1_{q}", tag=f"w1_{q}")
                for q in range(4)
            ]
            rep_sb = weights.tile([128, 1024], f16)
            w2_sb = [
                weights.tile([128, 2048], f16, name=f"w2_{pb}", tag=f"w2_{pb}")
                for pb in range(8)
            ]
            cpk = consts.tile([128, 8], f32)
            iota_b = consts.tile([128, S], f16)
            c0row = consts.tile([1, T], f16)
            onerow = consts.tile([1, 128], f16)
            bdep_c = cpk[:, 0:1]
            bcomp_c = cpk[:, 1:2]
            base_c = cpk[:, 2:3]
            headsf_t = [cpk[:, 3:4], cpk[:, 4:5]]
            wr_t = [cpk[:, 5:6], cpk[:, 6:7]]

            # ---- DMAs: strict consumption order, two HW DGE queues ----
            # sync queue
            nc.sync.dma_start(out=tok16_sb[:, 0:1024], in_=tok16_d[0, :, :])
            nc.sync.dma_start(out=tok16_sb[:, 1024:2048], in_=tok16_d[1, :, :])
            nc.sync.dma_start(out=w1_sb[0], in_=w1_d[0, :, :])
            nc.sync.dma_start(out=w1_sb[1], in_=w1_d[1, :, :])
            for pb in (0, 2, 4, 6, 5, 7):
                nc.sync.dma_start(out=w2_sb[pb], in_=w2_d[pb, :, :])
            nc.sync.dma_start(out=c0row, in_=c0_d[:, :])
            nc.sync.dma_start(out=onerow, in_=one_d[:, :])
            # scalar queue
            nc.scalar.dma_start(out=dep8_sb, in_=dep8_d[:, :])
            nc.scalar.dma_start(out=tok16_sb[:, 2048:3072], in_=tok16_d[2, :, :])
            nc.scalar.dma_start(out=tok16_sb[:, 3072:4096], in_=tok16_d[3, :, :])
            nc.scalar.dma_start(out=w1_sb[2], in_=w1_d[2, :, :])
            nc.scalar.dma_start(out=w1_sb[3], in_=w1_d[3, :, :])
            nc.scalar.dma_start(out=cpk, in_=cpk_d[:, :])
            nc.scalar.dma_start(out=rep_sb, in_=rep_d[:, :])
            for pb in (1, 3):
                nc.scalar.dma_start(out=w2_sb[pb], in_=w2_d[pb, :, :])
            nc.scalar.dma_start(out=iota_b, in_=bcast_row(iota_d[:, :]))

            # preload tanh act table off the critical path
            warm = consts.tile([128, 1], f32)
            nc.scalar.activation(warm, cpk[:, 7:8], Act.Tanh)

            # ---- stage 1: tde[p, j], 64 K-chunks, one fused U op per q ----
            tde_ps = pstde.tile([128, S], f32)
            ci1 = 0
            for q in range(4):
                # U_q[p, (db, ti, j)] = tok16[p, (q, ti, j)] * dep8[p, (db, j)]
                u = up.tile([128, 4096], f16, name="U", tag="U")
                out_v = u[:, :].rearrange("p (db ti j) -> p db ti j", ti=4, j=256)
                in0_v = tok16_sb[:, q * 1024:(q + 1) * 1024] \
                    .rearrange("p (ti j) -> p ti j", j=256) \
                    .unsqueeze(1).broadcast_to([128, 4, 4, 256])
                in1_v = dep8_sb[:, :].rearrange("p (db j) -> p db j", j=256) \
                    .unsqueeze(2).broadcast_to([128, 4, 4, 256])
                nc.vector.tensor_tensor(out=out_v, in0=in0_v, in1=in1_v,
                                        op=Alu.mult)
                for db in range(4):
                    for ti in range(4):
                        nc.tensor.matmul(
                            tde_ps,
                            w1_sb[q][:, (db * 4 + ti) * 128:(db * 4 + ti + 1) * 128],
                            u[:, (db * 4 + ti) * 256:(db * 4 + ti + 1) * 256],
                            start=(ci1 == 0),
                            stop=(ci1 == 63),
                        )
                        ci1 += 1

            # h = tanh(tde + b_dep), f16 [p, j]
            hT = acts.tile([128, S], f16)
            nc.scalar.activation(hT, tde_ps, Act.Tanh, bias=bdep_c)

            # ---- stage 2: comp[o, j], 128 K-chunks, one fused V op per pb ----
            comp_ps = pscomp.tile([128, S], f32)
            rep_ps = {}

            def emit_rep(pb):
                ps = psrep.tile([128, S], f32, name="rep_ps", tag="rep_ps")
                nc.tensor.matmul(
                    ps, rep_sb[:, pb * 128:(pb + 1) * 128], hT,
                    start=True, stop=True,
                )
                rep_ps[pb] = ps

            emit_rep(0)
            for pb in range(8):
                if pb + 1 < 8:
                    emit_rep(pb + 1)
                h8 = h8p.tile([128, S], f16, name="h8", tag="h8")
                nc.scalar.copy(h8, rep_ps.pop(pb))
                v = vp.tile([128, 4096], f16, name="V", tag="V")
                out_v = v[:, :].rearrange("p (tb j) -> p tb j", j=256)
                in0_v = tok16_sb[:, :].rearrange("p (tb j) -> p tb j", j=256)
                in1_v = h8[:, :].unsqueeze(1).broadcast_to([128, 16, 256])
                nc.vector.tensor_tensor(out=out_v, in0=in0_v, in1=in1_v,
                                        op=Alu.mult)
                for tb in range(16):
                    ci2 = pb * 16 + tb
                    nc.tensor.matmul(
                        comp_ps,
                        w2_sb[pb][:, tb * 128:(tb + 1) * 128],
                        v[:, tb * 256:(tb + 1) * 256],
                        start=(ci2 == 0),
                        stop=(ci2 == 127),
                    )

            # soh[jt][j, i] = (head[j] == i) * wr[j], f16
            soh = []
            for jt in range(JT):
                s = acts.tile([128, S], f16, name=f"soh{jt}", tag=f"soh{jt}")
                nc.vector.tensor_scalar(
                    out=s, in0=iota_b, scalar1=headsf_t[jt], scalar2=wr_t[jt],
                    op0=Alu.is_equal, op1=Alu.mult,
                )
                soh.append(s)

            # spec = tanh(comp + b_comp); delta = spec - base  (f16, [o, j])
            specT = acts.tile([128, S], f32)
            nc.scalar.activation(specT, comp_ps, Act.Tanh, bias=bcomp_c)
            deltaT = acts.tile([128, S], f16)
            nc.vector.tensor_scalar(
                out=deltaT, in0=specT, scalar1=base_c, scalar2=None,
                op0=Alu.subtract,
            )

            # transpose deltaT -> delta[j, o] per token tile
            delta_sb = []
            for jt in range(JT):
                dps = psmm.tile([128, 128], f16, name="dps", tag="dps")
                nc.tensor.transpose(
                    dps, deltaT[:, jt * 128:(jt + 1) * 128], ident16
                )
                dsb = acts.tile([128, 128], f16, name=f"delta{jt}", tag=f"delta{jt}")
                nc.scalar.copy(dsb, dps)
                delta_sb.append(dsb)

            # fin[i, o] = sum_jt soh[jt][:, i-chunk].T @ delta[jt]  (+ c0 via
            # a K=1 rank-1 matmul: ones[1,i] x c0row[1,o])
            fin_ps = psfin.tile([128, S], f32)
            for ic in range(2):
                for jt in range(JT):
                    nc.tensor.matmul(
                        fin_ps[:, ic * 128:(ic + 1) * 128],
                        soh[jt][:, ic * 128:(ic + 1) * 128],
                        delta_sb[jt],
                        start=(jt == 0),
                        stop=False,
                    )
                nc.tensor.matmul(
                    fin_ps[:, ic * 128:(ic + 1) * 128],
                    onerow, c0row,
                    start=False, stop=True,
                )
            for ic in range(2):
                outsb = acts.tile([128, T], f32, name="outsb", tag=f"outsb{ic}")
                nc.scalar.copy(outsb, fin_ps[:, ic * 128:(ic + 1) * 128])
                nc.sync.dma_start(
                    out=out_d[ic * 128:(ic + 1) * 128, :], in_=outsb
                )

    _NC_CACHE["nc"] = nc
    return nc


def prep_core_inputs(token_embeddings, dep_embeddings, dep_heads,
                     W_dep, b_dep, W_comp, b_comp, W_red, b_red):
    f32 = np.float32
    f16 = np.float16
    tok = np.asarray(token_embeddings, dtype=f32)
    dep = np.asarray(dep_embeddings, dtype=f32)
    heads = np.asarray(dep_heads)
    W_dep = np.asarray(W_dep, dtype=f32)
    b_dep = np.asarray(b_dep, dtype=f32)
    W_comp = np.asarray(W_comp, dtype=f32)
    b_comp = np.asarray(b_comp, dtype=f32)
    wr = np.asarray(W_red, dtype=f32)[0]
    b_red = np.asarray(b_red, dtype=f32)

    # W1sb[(d',t'), (q, db, ti, p)] = W_dep[p, 8*(4q+ti)+t', 16*db+d']
    A = W_dep.reshape(P, 4, 4, 8, 4, 16)          # [p, q, ti, t', db, d']
    w1 = np.ascontiguousarray(
        A.transpose(5, 3, 1, 4, 2, 0).reshape(128, 4, 2048).transpose(1, 0, 2)
    ).astype(f16)                                 # [q, 128, 2048]

    # W2sb[(p',t'), (pb, tb, o)] = W_comp[o, 8*tb+t', 16*pb+p']
    Bm = W_comp.reshape(T, 16, 8, 8, 16)          # [o, tb, t', pb, p']
    w2 = np.ascontiguousarray(
        Bm.transpose(4, 2, 3, 1, 0).reshape(128, 8, 2048).transpose(1, 0, 2)
    ).astype(f16)                                 # [pb, 128, 2048]

    # rep[k, (pb, r)] = 1 if k == 16*pb + r//8
    rep = np.zeros((128, 8, 128), dtype=f16)
    r_ = np.arange(128)
    for pb in range(8):
        rep[16 * pb + r_ // 8, pb, r_] = 1.0
    rep = rep.reshape(128, 1024)

    base = np.tanh(b_comp)
    c0 = (base * wr.sum() + b_red[0]).astype(f32)
    iota = np.arange(S, dtype=f16).reshape(1, S)
    headsf = heads.astype(f32).reshape(B, JT, 128)
    wr_t = np.ascontiguousarray(wr.reshape(JT, 128, 1))

    cpk = np.zeros((128, 8), dtype=f32)
    cpk[:, 0] = b_dep
    cpk[:, 1] = b_comp
    cpk[:, 2] = base
    cpk[:, 5] = wr[:128]
    cpk[:, 6] = wr[128:]
    shared = {
        "w1": w1, "w2": w2, "rep": rep,
        "iota": iota,
        "c0": c0.reshape(1, T).astype(f16),
        "one": np.ones((1, 128), dtype=f16),
    }
    in_maps = []
    for c in range(NCORES):
        # tok16[(rep16, t'), (tb, j)] = tok[c][j, 8*tb + t']
        tokT3 = np.ascontiguousarray(tok[c].T).reshape(16, 8, S)   # [tb, t', j]
        tmp = tokT3.transpose(1, 0, 2)                             # [t', tb, j]
        tok16 = np.ascontiguousarray(
            np.broadcast_to(tmp[None], (16, 8, 16, S))
            .reshape(128, 4, 1024).transpose(1, 0, 2)
        ).astype(f16)                                              # [q, 128, 1024]
        # dep8[(d', rep8), (db, j)] = dep[c][j, 16*db + d']
        depT3 = np.ascontiguousarray(dep[c].T).reshape(4, 16, S)   # [db, d', j]
        dmp = depT3.transpose(1, 0, 2)                             # [d', db, j]
        dep8 = np.ascontiguousarray(
            np.broadcast_to(dmp[:, None], (16, 8, 4, S)).reshape(128, 4 * S)
        ).astype(f16)
        m = dict(shared)
        m["tok16"] = tok16
        m["dep8"] = dep8
        cpkc = cpk.copy()
        cpkc[:, 3] = headsf[c, 0]
        cpkc[:, 4] = headsf[c, 1]
        m["cpk"] = cpkc
        in_maps.append(m)
    return in_maps


def kernel(**inputs) -> np.ndarray:
    _install_compat()
    from concourse.bass_utils import run_bass_kernel_spmd

    nc = build_nc()
    in_maps = prep_core_inputs(**inputs)
    res = run_bass_kernel_spmd(nc, in_maps, core_ids=list(range(NCORES)))
    out = np.stack([res.results[c]["out"] for c in range(NCORES)], axis=0)
    return out.astype(np.float32)


# aliases used by test harness
_build_nc = build_nc
_prep_core_inputs = prep_core_inputs


# revision 17
# speedup vs baseline: 1.3384x; 1.0448x over previous
"""Trainium2 Bass kernel for nn_CompositionBlock (gnn_message_passing).

Data-parallel over batch B=8 across 8 NeuronCores (one sample per core).

U-first bilinear formulation: for each token j the bilinears are
  tde[p,j] = sum_{t,d} W1[p,t,d] * (tok[j,t]*dep[j,d])
  comp[o,j] = sum_{t,p} W2[o,t,p] * (tok[j,t]*h[p,j])
The elementwise outer-product operands U/V are built by DVE from
host-replicated f16 tiles (all-SBUF 16-bit => DVE fast mode), and each
bilinear is ONE PSUM accumulation chain of K-chunk matmuls (no selection
-matrix reduction matmuls).  h replication across partitions is done on
the PE with a selection matrix.  Head-scatter stays a one-hot matmul.
"""

import json

import numpy as np

B, S, T, D, P = 8, 256, 128, 64, 128
NCORES = 8
JT = S // 128  # token tiles (j) per core


# ----------------------------------------------------------------------------
# Compat: the walrus build in this container accepts at most one sync-wait on
# CTRL-class instructions, but TileContext's tail drain packs several. Split
# any multi-wait instruction into a chain of single-wait clones.
# ----------------------------------------------------------------------------
def _split_multiwait_bir(bir_json_bytes: bytes) -> bytes:
    bir = json.loads(bir_json_bytes)
    for func in bir.get("functions", []):
        for bb in func.get("blocks", []):
            new_instructions = []
            for ins in bb.get("instructions", []):
                si = ins.get("sync_info") or {}
                waits = si.get("on_wait") or []
                if len(waits) > 1:
                    for i, w in enumerate(waits[:-1]):
                        new_instructions.append({
                            "debug": ins.get("debug", 0),
                            "engine": ins["engine"],
                            "ins": [],
                            "name": f"{ins['name']}_w{i}",
                            "opcode": "NoOp",
                            "outs": [],
                            "sync_info": {"on_wait": [w], "on_update": []},
                        })
                    ins["sync_info"] = {
                        "on_wait": [waits[-1]],
                        "on_update": si.get("on_update") or [],
                    }
                new_instructions.append(ins)
            bb["instructions"] = new_instructions
    return json.dumps(bir).encode()


def _install_compat():
    import concourse.bass_utils as bu

    if getattr(bu.compile_bir_kernel, "_multiwait_patched", False):
        return
    orig = bu.compile_bir_kernel

    def patched(bir_json, tmpdir, neff_name="file.neff"):
        return orig(_split_multiwait_bir(bir_json), tmpdir, neff_name)

    patched._multiwait_patched = True
    bu.compile_bir_kernel = patched
    try:
        import concourse.bass2jax as b2j

        if getattr(b2j, "compile_bir_kernel", None) is not None:
            b2j.compile_bir_kernel = patched
    except ImportError:
        pass


_NC_CACHE = {}


def build_nc():
    if "nc" in _NC_CACHE:
        return _NC_CACHE["nc"]
    import concourse.bass as bass
    import concourse.tile as tile
    from concourse import mybir
    from concourse.masks import make_identity

    f32 = mybir.dt.float32
    f16 = mybir.dt.float16
    Alu = mybir.AluOpType
    Act = mybir.ActivationFunctionType

    nc = bass.Bass(trn_type="TRN2")

    # DRAM inputs (all host-prepped into final SBUF layouts)
    tok16_d = nc.dram_tensor("tok16", [4, 128, 1024], f16, kind="ExternalInput")
    dep8_d = nc.dram_tensor("dep8", [128, 1024], f16, kind="ExternalInput")
    w1_d = nc.dram_tensor("w1", [4, 128, 2048], f16, kind="ExternalInput")
    w2_d = nc.dram_tensor("w2", [8, 128, 2048], f16, kind="ExternalInput")
    rep_d = nc.dram_tensor("rep", [128, 1024], f16, kind="ExternalInput")
    iota_d = nc.dram_tensor("iota", [128, S], f16, kind="ExternalInput")
    cpk_d = nc.dram_tensor("cpk", [128, 8], f32, kind="ExternalInput")
    c0_d = nc.dram_tensor("c0", [1, T], f16, kind="ExternalInput")
    one_d = nc.dram_tensor("one", [1, 128], f16, kind="ExternalInput")
    out_d = nc.dram_tensor("out", [S, T], f32, kind="ExternalOutput")

    def bcast_row(dram_ap):
        return bass.AP(
            tensor=dram_ap.tensor,
            offset=dram_ap.offset,
            ap=[[0, 128]] + list(dram_ap.ap[1:]),
        )

    with tile.TileContext(nc) as tc:
        with (
            tc.tile_pool(name="consts", bufs=1) as consts,
            tc.tile_pool(name="weights", bufs=1) as weights,
            tc.tile_pool(name="acts", bufs=1) as acts,
            tc.tile_pool(name="up", bufs=3) as up,
            tc.tile_pool(name="h8p", bufs=3) as h8p,
            tc.tile_pool(name="vp", bufs=3) as vp,
            tc.tile_pool(name="pstde", bufs=1, space="PSUM") as pstde,
            tc.tile_pool(name="psrep", bufs=2, space="PSUM") as psrep,
            tc.tile_pool(name="pscomp", bufs=1, space="PSUM") as pscomp,
            tc.tile_pool(name="psmm", bufs=2, space="PSUM") as psmm,
            tc.tile_pool(name="psfin", bufs=1, space="PSUM") as psfin,
        ):
            ident16 = consts.tile([128, 128], f16)
            make_identity(nc, ident16)

            # ---- tiles ----
            tok16_sb = weights.tile([128, 4096], f16)
            dep8_sb = weights.tile([128, 1024], f16)
            w1_sb = [
                weights.tile([128, 2048], f16, name=f"w1_{q}", tag=f"w1_{q}")
                for q in range(4)
            ]
            rep_sb = weights.tile([128, 1024], f16)
            w2_sb = [
                weights.tile([128, 2048], f16, name=f"w2_{pb}", tag=f"w2_{pb}")
                for pb in range(8)
            ]
            cpk = consts.tile([128, 8], f32)
            iota_b = consts.tile([128, S], f16)
            c0row = consts.tile([1, T], f16)
            onerow = consts.tile([1, 128], f16)
            bdep_c = cpk[:, 0:1]
            bcomp_c = cpk[:, 1:2]
            base_c = cpk[:, 2:3]
            headsf_t = [cpk[:, 3:4], cpk[:, 4:5]]
            wr_t = [cpk[:, 5:6], cpk[:, 6:7]]

            # ---- DMAs: strict consumption order, two HW DGE queues ----
            # sync queue
            nc.sync.dma_start(out=tok16_sb[:, 0:1024], in_=tok16_d[0, :, :])
            nc.sync.dma_start(out=w1_sb[0], in_=w1_d[0, :, :])
            nc.sync.dma_start(out=tok16_sb[:, 1024:2048], in_=tok16_d[1, :, :])
            nc.sync.dma_start(out=w1_sb[1], in_=w1_d[1, :, :])
            nc.sync.dma_start(out=tok16_sb[:, 2048:3072], in_=tok16_d[2, :, :])
            for pb in (0, 2, 4, 6):
                nc.sync.dma_start(out=w2_sb[pb], in_=w2_d[pb, :, :])
            nc.sync.dma_start(out=c0row, in_=c0_d[:, :])
            nc.sync.dma_start(out=onerow, in_=one_d[:, :])
            # scalar queue
            nc.scalar.dma_start(out=dep8_sb, in_=dep8_d[:, :])
            nc.scalar.dma_start(out=iota_b, in_=iota_d[:, :])
            nc.scalar.dma_start(out=tok16_sb[:, 3072:4096], in_=tok16_d[3, :, :])
            nc.scalar.dma_start(out=w1_sb[2], in_=w1_d[2, :, :])
            nc.scalar.dma_start(out=w1_sb[3], in_=w1_d[3, :, :])
            nc.scalar.dma_start(out=cpk, in_=cpk_d[:, :])
            nc.scalar.dma_start(out=rep_sb, in_=rep_d[:, :])
            for pb in (1, 3, 5, 7):
                nc.scalar.dma_start(out=w2_sb[pb], in_=w2_d[pb, :, :])

            # preload tanh act table off the critical path
            warm = consts.tile([128, 1], f32)
            nc.scalar.activation(warm, cpk[:, 7:8], Act.Tanh)

            # ---- stage 1: tde[p, j], 64 K-chunks, one fused U op per q ----
            tde_ps = pstde.tile([128, S], f32)
            ci1 = 0
            for q in range(4):
                # U_q[p, (db, ti, j)] = tok16[p, (q, ti, j)] * dep8[p, (db, j)]
                u = up.tile([128, 4096], f16, name="U", tag="U")
                out_v = u[:, :].rearrange("p (db ti j) -> p db ti j", ti=4, j=256)
                in0_v = tok16_sb[:, q * 1024:(q + 1) * 1024] \
                    .rearrange("p (ti j) -> p ti j", j=256) \
                    .unsqueeze(1).broadcast_to([128, 4, 4, 256])
                in1_v = dep8_sb[:, :].rearrange("p (db j) -> p db j", j=256) \
                    .unsqueeze(2).broadcast_to([128, 4, 4, 256])
                nc.vector.tensor_tensor(out=out_v, in0=in0_v, in1=in1_v,
                                        op=Alu.mult)
                for db in range(4):
                    for ti in range(4):
                        nc.tensor.matmul(
                            tde_ps,
                            w1_sb[q][:, (db * 4 + ti) * 128:(db * 4 + ti + 1) * 128],
                            u[:, (db * 4 + ti) * 256:(db * 4 + ti + 1) * 256],
                            start=(ci1 == 0),
                            stop=(ci1 == 63),
                        )
                        ci1 += 1

            # h = tanh(tde + b_dep), f16 [p, j]
            hT = acts.tile([128, S], f16)
            nc.scalar.activation(hT, tde_ps, Act.Tanh, bias=bdep_c)

            # ---- stage 2: comp[o, j], 128 K-chunks, one fused V op per pb ----
            comp_ps = pscomp.tile([128, S], f32)
            rep_ps = {}

            def emit_rep(pb):
                ps = psrep.tile([128, S], f32, name="rep_ps", tag="rep_ps")
                nc.tensor.matmul(
                    ps, rep_sb[:, pb * 128:(pb + 1) * 128], hT,
                    start=True, stop=True,
                )
                rep_ps[pb] = ps

            emit_rep(0)
            for pb in range(8):
                if pb + 1 < 8:
                    emit_rep(pb + 1)
                h8 = h8p.tile([128, S], f16, name="h8", tag="h8")
                nc.scalar.copy(h8, rep_ps.pop(pb))
                v = vp.tile([128, 4096], f16, name="V", tag="V")
                out_v = v[:, :].rearrange("p (tb j) -> p tb j", j=256)
                in0_v = tok16_sb[:, :].rearrange("p (tb j) -> p tb j", j=256)
                in1_v = h8[:, :].unsqueeze(1).broadcast_to([128, 16, 256])
                nc.vector.tensor_tensor(out=out_v, in0=in0_v, in1=in1_v,
                                        op=Alu.mult)
                for tb in range(16):
                    ci2 = pb * 16 + tb
                    nc.tensor.matmul(
                        comp_ps,
                        w2_sb[pb][:, tb * 128:(tb + 1) * 128],
                        v[:, tb * 256:(tb + 1) * 256],
                        start=(ci2 == 0),
                        stop=(ci2 == 127),
                    )

            # soh[jt][j, i] = (head[j] == i) * wr[j], f16
            soh = []
            for jt in range(JT):
                s = acts.tile([128, S], f16, name=f"soh{jt}", tag=f"soh{jt}")
                nc.vector.tensor_scalar(
                    out=s, in0=iota_b, scalar1=headsf_t[jt], scalar2=wr_t[jt],
                    op0=Alu.is_equal, op1=Alu.mult,
                )
                soh.append(s)

            # spec = tanh(comp + b_comp); delta = spec - base  (f16, [o, j])
            specT = acts.tile([128, S], f32)
            nc.scalar.activation(specT, comp_ps, Act.Tanh, bias=bcomp_c)
            deltaT = acts.tile([128, S], f16)
            nc.vector.tensor_scalar(
                out=deltaT, in0=specT, scalar1=base_c, scalar2=None,
                op0=Alu.subtract,
            )

            # transpose deltaT -> delta[j, o] per token tile
            delta_sb = []
            for jt in range(JT):
                dps = psmm.tile([128, 128], f16, name="dps", tag="dps")
                nc.tensor.transpose(
                    dps, deltaT[:, jt * 128:(jt + 1) * 128], ident16
                )
                dsb = acts.tile([128, 128], f16, name=f"delta{jt}", tag=f"delta{jt}")
                nc.scalar.copy(dsb, dps)
                delta_sb.append(dsb)

            # fin[i, o] = sum_jt soh[jt][:, i-chunk].T @ delta[jt]  (+ c0 via
            # a K=1 rank-1 matmul: ones[1,i] x c0row[1,o])
            fin_ps = psfin.tile([128, S], f32)
            for ic in range(2):
                for jt in range(JT):
                    nc.tensor.matmul(
                        fin_ps[:, ic * 128:(ic + 1) * 128],
                        soh[jt][:, ic * 128:(ic + 1) * 128],
                        delta_sb[jt],
                        start=(jt == 0),
                        stop=False,
                    )
                nc.tensor.matmul(
                    fin_ps[:, ic * 128:(ic + 1) * 128],
                    onerow, c0row,
                    start=False, stop=True,
                )
            for ic in range(2):
                outsb = acts.tile([128, T], f32, name="outsb", tag=f"outsb{ic}")
                nc.scalar.copy(outsb, fin_ps[:, ic * 128:(ic + 1) * 128])
                nc.sync.dma_start(
                    out=out_d[ic * 128:(ic + 1) * 128, :], in_=outsb
                )

    _NC_CACHE["nc"] = nc
    return nc


def prep_core_inputs(token_embeddings, dep_embeddings, dep_heads,
                     W_dep, b_dep, W_comp, b_comp, W_red, b_red):
    f32 = np.float32
    f16 = np.float16
    tok = np.asarray(token_embeddings, dtype=f32)
    dep = np.asarray(dep_embeddings, dtype=f32)
    heads = np.asarray(dep_heads)
    W_dep = np.asarray(W_dep, dtype=f32)
    b_dep = np.asarray(b_dep, dtype=f32)
    W_comp = np.asarray(W_comp, dtype=f32)
    b_comp = np.asarray(b_comp, dtype=f32)
    wr = np.asarray(W_red, dtype=f32)[0]
    b_red = np.asarray(b_red, dtype=f32)

    # W1sb[(d',t'), (q, db, ti, p)] = W_dep[p, 8*(4q+ti)+t', 16*db+d']
    A = W_dep.reshape(P, 4, 4, 8, 4, 16)          # [p, q, ti, t', db, d']
    w1 = np.ascontiguousarray(
        A.transpose(5, 3, 1, 4, 2, 0).reshape(128, 4, 2048).transpose(1, 0, 2)
    ).astype(f16)                                 # [q, 128, 2048]

    # W2sb[(p',t'), (pb, tb, o)] = W_comp[o, 8*tb+t', 16*pb+p']
    Bm = W_comp.reshape(T, 16, 8, 8, 16)          # [o, tb, t', pb, p']
    w2 = np.ascontiguousarray(
        Bm.transpose(4, 2, 3, 1, 0).reshape(128, 8, 2048).transpose(1, 0, 2)
    ).astype(f16)                                 # [pb, 128, 2048]

    # rep[k, (pb, r)] = 1 if k == 16*pb + r//8
    rep = np.zeros((128, 8, 128), dtype=f16)
    r_ = np.arange(128)
    for pb in range(8):
        rep[16 * pb + r_ // 8, pb, r_] = 1.0
    rep = rep.reshape(128, 1024)

    base = np.tanh(b_comp)
    c0 = (base * wr.sum() + b_red[0]).astype(f32)
    iota = np.ascontiguousarray(
        np.broadcast_to(np.arange(S, dtype=f16), (128, S)))
    headsf = heads.astype(f32).reshape(B, JT, 128)
    wr_t = np.ascontiguousarray(wr.reshape(JT, 128, 1))

    cpk = np.zeros((128, 8), dtype=f32)
    cpk[:, 0] = b_dep
    cpk[:, 1] = b_comp
    cpk[:, 2] = base
    cpk[:, 5] = wr[:128]
    cpk[:, 6] = wr[128:]
    shared = {
        "w1": w1, "w2": w2, "rep": rep,
        "iota": iota,
        "c0": c0.reshape(1, T).astype(f16),
        "one": np.ones((1, 128), dtype=f16),
    }
    in_maps = []
    for c in range(NCORES):
        # tok16[(rep16, t'), (tb, j)] = tok[c][j, 8*tb + t']
        tokT3 = np.ascontiguousarray(tok[c].T).reshape(16, 8, S)   # [tb, t', j]
        tmp = tokT3.transpose(1, 0, 2)                             # [t', tb, j]
        tok16 = np.ascontiguousarray(
            np.broadcast_to(tmp[None], (16, 8, 16, S))
            .reshape(128, 4, 1024).transpose(1, 0, 2)
        ).astype(f16)                                              # [q, 128, 1024]
        # dep8[(d', rep8), (db, j)] = dep[c][j, 16*db + d']
        depT3 = np.ascontiguousarray(dep[c].T).reshape(4, 16, S)   # [db, d', j]
        dmp = depT3.transpose(1, 0, 2)                             # [d', db, j]
        dep8 = np.ascontiguousarray(
            np.broadcast_to(dmp[:, None], (16, 8, 4, S)).reshape(128, 4 * S)
        ).astype(f16)
        m = dict(shared)
        m["tok16"] = tok16
        m["dep8"] = dep8
        cpkc = cpk.copy()
        cpkc[:, 3] = headsf[c, 0]
        cpkc[:, 4] = headsf[c, 1]
        m["cpk"] = cpkc
        in_maps.append(m)
    return in_maps


def kernel(**inputs) -> np.ndarray:
    _install_compat()
    from concourse.bass_utils import run_bass_kernel_spmd

    nc = build_nc()
    in_maps = prep_core_inputs(**inputs)
    res = run_bass_kernel_spmd(nc, in_maps, core_ids=list(range(NCORES)))
    out = np.stack([res.results[c]["out"] for c in range(NCORES)], axis=0)
    return out.astype(np.float32)


# aliases used by test harness
_build_nc = build_nc
_prep_core_inputs = prep_core_inputs
